# revision 1
# baseline (speedup 1.0000x reference)
"""Linear-attention head (elu+1 feature map) on 8 TRN2 NeuronCores.

Pure data parallel: batch 16 -> 2 batches per core; the three 1024x1024
projection weights are replicated. Everything on-device runs in transposed
space [feature, seq] so the projection outputs feed the two chained
matmuls without any transposes:

  kt[d,s] = Wk @ x^T           (phi_k^T after elu+1, padding forced to 0
                                via a rank-1 -1e9 row added in PSUM)
  vt[d,s] = (Wv @ x^T + bv)*keep
  qt[d,s] = phi_q^T (unmasked; mask folded into final scale)
  A[i,j]  = sum_d vt[d,i]*kt[d,j]          == kv[i,j]
  O[s,j]  = sum_i qt[i,s]*A[i,j]
  den[s]  = sum_i qt[i,s]*ksum[i]          (ksum = free-dim accum of kt)
  out     = O * keep[s] / max(den, eps)

Matmuls run as float32r (fp32 storage, FP22 multiply) at full rate.
elu(x)+1 is computed exactly as min(exp(x),1) + relu(x).

Host-side work is layout marshalling only (transposes / reshapes).
"""

import sys

import numpy as np

if "/opt/trn_rl_repo" not in sys.path:
    sys.path.insert(0, "/opt/trn_rl_repo")

B, S, DM, DH = 16, 1024, 1024, 1024
NCORES = 8
BPC = B // NCORES  # batches per core
P = 128
NT = S // P  # 8 tiles of 128
NEG = -1.0e9
EPS = 1e-6

_CACHE = {}


def _build_nc():
    import concourse.bacc as bacc
    import concourse.bass as bass
    import concourse.mybir as mybir
    import concourse.tile as tile

    f32 = mybir.dt.float32
    f32r = mybir.dt.float32r
    Act = mybir.ActivationFunctionType
    Op = mybir.AluOpType

    nc = bacc.Bacc()

    xt_ext = nc.declare_dram_parameter("xt", [BPC, DM, S], f32, isOutput=False)
    wt_ext = {
        "q": nc.declare_dram_parameter("wqt", [DM, DH], f32, isOutput=False),
        "k": nc.declare_dram_parameter("wkt", [DM, DH], f32, isOutput=False),
        "v": nc.declare_dram_parameter("wvt", [DM, DH], f32, isOutput=False),
    }
    bias_ext = nc.declare_dram_parameter("bias", [P, 3 * NT], f32, isOutput=False)
    consts_ext = nc.declare_dram_parameter("consts", [2, P], f32, isOutput=False)
    mrow_ext = nc.declare_dram_parameter("mrow", [BPC, S], f32, isOutput=False)
    mcol_ext = nc.declare_dram_parameter("mcol", [BPC, P, NT], f32, isOutput=False)
    out_ext = nc.declare_dram_parameter("out", [BPC, S, DH], f32, isOutput=True)

    BIAS_COL = {"q": 0, "k": NT, "v": 2 * NT}

    def r(ap):
        return ap.bitcast(f32r)

    with tile.TileContext(nc) as tc:
        with (
            tc.tile_pool(name="const", bufs=1) as cpool,
            tc.tile_pool(name="rows", bufs=1) as rpool,
            tc.tile_pool(name="keept", bufs=1) as ktpool,
            tc.tile_pool(name="tiny", bufs=2) as spool,
            tc.tile_pool(name="xt", bufs=8) as xtpool,
            tc.tile_pool(name="at", bufs=8) as atpool,
            tc.tile_pool(name="kvq", bufs=8) as kvqpool,
            tc.tile_pool(name="wt", bufs=4) as wpool,
            tc.tile_pool(name="actE", bufs=2) as apool,
            tc.tile_pool(name="actR", bufs=1) as rrpool,
            tc.tile_pool(name="ost", bufs=2) as opool,
            tc.tile_pool(name="ps", bufs=3, space="PSUM") as pspool,
            tc.tile_pool(name="psden", bufs=2, space="PSUM") as dpool,
        ):
            # ---- constants ----
            bias_sb = cpool.tile([P, 3 * NT], f32, tag="bias")
            nc.sync.dma_start(bias_sb[:], bias_ext[:, :])
            ones_col = cpool.tile([1, P], f32, tag="ones")
            nc.sync.dma_start(r(ones_col[:]), r(consts_ext[0:1, :]))
            neg_col = cpool.tile([1, P], f32, tag="neg")
            nc.sync.dma_start(r(neg_col[:]), r(consts_ext[1:2, :]))

            def fence(reads, writes):
                # walrus' Matmult pseudo carries at most ONE embedded sync
                # wait. A PE NoOp declaring the group's reads/writes absorbs
                # all foreign-proc waits (NoOp carries many, like the Tile
                # tail drain), leaving each matmul's own wait count <= 1.
                eng = nc.tensor
                eng.add_instruction(
                    mybir.InstNoOp(
                        name=nc.get_next_instruction_name(),
                        text_hint="dep_fence",
                        bass_nofuse=True,
                        ins=[eng.lower_ap(a) for a in reads],
                        outs=[eng.lower_ap(a) for a in writes],
                    )
                )

            def mm_psum(reads):
                ps = pspool.tile([P, S], f32, tag="mm")
                fence(reads, [ps[:]])
                return ps

            for b in range(BPC):
                # ---- mask prep ----
                mrow = rpool.tile([1, S], f32, tag="mrow")
                nc.sync.dma_start(r(mrow[:]), r(mrow_ext[b : b + 1, :]))
                mcol = spool.tile([P, NT], f32, tag="mcol")
                nc.sync.dma_start(mcol[:], mcol_ext[b])
                keepcol = spool.tile([P, NT], f32, tag="keepcol")
                nc.vector.tensor_scalar(
                    out=keepcol[:], in0=mcol[:], scalar1=-1.0, scalar2=1.0,
                    op0=Op.mult, op1=Op.add,
                )
                # broadcast mrow to all 128 partitions via PE rank-1, then
                # flip (keep = 1 - pad) during the PSUM evacuation
                kb_ps = mm_psum([ones_col[:], mrow[:]])
                for c in range(2):
                    cs = slice(c * 512, (c + 1) * 512)
                    nc.tensor.matmul(
                        kb_ps[:, cs], r(ones_col[:]), r(mrow[:, cs]),
                        start=True, stop=True,
                    )
                keep_tile = ktpool.tile([P, S], f32, tag="keeptile")
                nc.vector.tensor_scalar(
                    out=keep_tile[:], in0=kb_ps[:], scalar1=-1.0, scalar2=1.0,
                    op0=Op.mult, op1=Op.add,
                )

                # ---- x^T tiles ----
                xt = []
                for mt in range(NT):
                    t = xtpool.tile([P, S], f32, tag="xt")
                    nc.sync.dma_start(r(t[:]), r(xt_ext[b, mt * P : (mt + 1) * P, :]))
                    xt.append(t)

                # ---- projections ----
                def project(which, masked_rank1, out_tag):
                    """Returns list of 8 [128(d), 1024(s)] PSUM tiles handed
                    one at a time to the epilogue callback via yield-like flow."""
                    tiles = []
                    for dt in range(NT):
                        wt = wpool.tile([P, NT * P], f32, tag="wt")
                        src = (
                            wt_ext[which][:, :]
                            .rearrange("(t p) d -> p t d", p=P)[
                                :, :, dt * P : (dt + 1) * P
                            ]
                        )
                        nc.gpsimd.dma_start(
                            r(wt[:].rearrange("p (t d) -> p t d", d=P)), r(src)
                        )
                        deps = [wt[:]] + [t[:] for t in xt]
                        if masked_rank1:
                            deps += [neg_col[:], mrow[:]]
                        ps = mm_psum(deps)
                        for c in range(2):
                            cs = slice(c * 512, (c + 1) * 512)
                            for mt in range(NT):
                                nc.tensor.matmul(
                                    ps[:, cs],
                                    r(wt[:, mt * P : (mt + 1) * P]),
                                    r(xt[mt][:, cs]),
                                    start=(mt == 0),
                                    stop=(mt == NT - 1) and not masked_rank1,
                                )
                            if masked_rank1:
                                nc.tensor.matmul(
                                    ps[:, cs], r(neg_col[:]), r(mrow[:, cs]),
                                    start=False, stop=True,
                                )
                        tiles.append(ps)
                    return tiles

                # K projection: rank-1 -1e9*pad row forces masked phi_k to 0
                kt = []
                ksum = spool.tile([P, NT + 1], f32, tag="ksum")
                for dt, ps in enumerate(project("k", True, "kt")):
                    bcol = bias_sb[:, BIAS_COL["k"] + dt : BIAS_COL["k"] + dt + 1]
                    E = apool.tile([P, S], f32, tag="E")
                    nc.scalar.activation(E[:], ps[:], Act.Exp, bias=bcol)
                    R = rrpool.tile([P, S], f32, tag="R")
                    nc.vector.tensor_scalar(
                        out=R[:], in0=ps[:], scalar1=bcol, scalar2=0.0,
                        op0=Op.add, op1=Op.max,
                    )
                    t = kvqpool.tile([P, S], f32, tag="kt")
                    nc.vector.scalar_tensor_tensor(
                        out=r(t[:]), in0=E[:], scalar=1.0, in1=R[:],
                        op0=Op.min, op1=Op.add,
                        accum_out=r(ksum[:, dt : dt + 1]),
                    )
                    kt.append(t)

                # V projection: (psum + bv) * keep
                vt = []
                for dt, ps in enumerate(project("v", False, "vt")):
                    bcol = bias_sb[:, BIAS_COL["v"] + dt : BIAS_COL["v"] + dt + 1]
                    t = kvqpool.tile([P, S], f32, tag="vt")
                    nc.vector.scalar_tensor_tensor(
                        out=r(t[:]), in0=ps[:], scalar=bcol, in1=keep_tile[:],
                        op0=Op.add, op1=Op.mult,
                    )
                    vt.append(t)

                # Q projection: unmasked phi_q (mask folded into final scale)
                qt = []
                for dt, ps in enumerate(project("q", False, "qt")):
                    bcol = bias_sb[:, BIAS_COL["q"] + dt : BIAS_COL["q"] + dt + 1]
                    E = apool.tile([P, S], f32, tag="E")
                    nc.scalar.activation(E[:], ps[:], Act.Exp, bias=bcol)
                    R = rrpool.tile([P, S], f32, tag="R")
                    nc.vector.tensor_scalar(
                        out=R[:], in0=ps[:], scalar1=bcol, scalar2=0.0,
                        op0=Op.add, op1=Op.max,
                    )
                    t = kvqpool.tile([P, S], f32, tag="qt")
                    nc.vector.scalar_tensor_tensor(
                        out=r(t[:]), in0=E[:], scalar=1.0, in1=R[:],
                        op0=Op.min, op1=Op.add,
                        # fp32r matmuls reject N=1; the denom matmuls run at
                        # N=2 with a pad column of ksum that must also be
                        # f32r-rounded data — fill it with a q-side accum.
                        accum_out=(
                            r(ksum[:, NT : NT + 1]) if dt == NT - 1 else None
                        ),
                    )
                    qt.append(t)

                # ---- A = V @ phi_k^T  (A[i,j], i=v row, j=phi_k row) ----
                at = []
                for it in range(NT):
                    ps = mm_psum([t[:] for t in vt] + [t[:] for t in kt])
                    for c in range(2):
                        cs = slice(c * 512, (c + 1) * 512)
                        for dt in range(NT):
                            nc.tensor.matmul(
                                ps[:, cs],
                                r(vt[dt][:, it * P : (it + 1) * P]),
                                r(kt[dt][:, cs]),
                                start=(dt == 0), stop=(dt == NT - 1),
                            )
                    t = atpool.tile([P, S], f32, tag="at")
                    nc.vector.tensor_copy(r(t[:]), ps[:])
                    at.append(t)

                # ---- O = phi_q @ A, denom, scale, store ----
                for st in range(NT):
                    ps = pspool.tile([P, S], f32, tag="mm")
                    dps = dpool.tile([P, 2], f32, tag="den")
                    fence(
                        [t[:] for t in qt] + [t[:] for t in at] + [ksum[:]],
                        [ps[:], dps[:]],
                    )
                    ss = slice(st * P, (st + 1) * P)
                    for c in range(2):
                        cs = slice(c * 512, (c + 1) * 512)
                        for it in range(NT):
                            nc.tensor.matmul(
                                ps[:, cs],
                                r(qt[it][:, ss]),
                                r(at[it][:, cs]),
                                start=(it == 0), stop=(it == NT - 1),
                            )
                    for it in range(NT):
                        nc.tensor.matmul(
                            dps[:],
                            r(qt[it][:, ss]),
                            r(ksum[:, it : it + 2]),
                            start=(it == 0), stop=(it == NT - 1),
                        )
                    dsb = spool.tile([P, 1], f32, tag="dsb")
                    nc.vector.tensor_scalar(
                        out=dsb[:], in0=dps[:, 0:1], scalar1=float(EPS), scalar2=None,
                        op0=Op.max,
                    )
                    z = spool.tile([P, 1], f32, tag="z")
                    nc.vector.reciprocal(z[:], dsb[:])
                    zm = spool.tile([P, 1], f32, tag="zm")
                    nc.vector.tensor_mul(zm[:], z[:], keepcol[:, st : st + 1])
                    o = opool.tile([P, S], f32, tag="ost")
                    nc.vector.tensor_scalar(
                        out=o[:], in0=ps[:], scalar1=zm[:], scalar2=None,
                        op0=Op.mult,
                    )
                    nc.sync.dma_start(out_ext[b, ss, :], o[:])

    nc.compile()
    return nc


def _prepare_in_maps(inputs):
    x = np.asarray(inputs["x"], np.float32)
    pm = np.asarray(inputs["padding_mask"])
    xt = np.ascontiguousarray(np.transpose(x, (0, 2, 1)))
    wqt = np.ascontiguousarray(np.asarray(inputs["Wq"], np.float32).T)
    wkt = np.ascontiguousarray(np.asarray(inputs["Wk"], np.float32).T)
    wvt = np.ascontiguousarray(np.asarray(inputs["Wv"], np.float32).T)
    bias = np.ascontiguousarray(
        np.concatenate(
            [
                np.asarray(inputs[k], np.float32).reshape(NT, P).T
                for k in ("bq", "bk", "bv")
            ],
            axis=1,
        )
    )
    mrow = np.ascontiguousarray((pm == 1).astype(np.float32))  # 1.0 = pad
    consts = np.ascontiguousarray(
        np.stack([np.ones(P, np.float32), np.full(P, NEG, np.float32)])
    )
    mcol = np.ascontiguousarray(mrow.reshape(B, NT, P).transpose(0, 2, 1))
    in_maps = []
    for i in range(NCORES):
        sl = slice(BPC * i, BPC * (i + 1))
        in_maps.append(
            {
                "xt": np.ascontiguousarray(xt[sl]),
                "wqt": wqt,
                "wkt": wkt,
                "wvt": wvt,
                "bias": bias,
                "consts": consts,
                "mrow": np.ascontiguousarray(mrow[sl]),
                "mcol": np.ascontiguousarray(mcol[sl]),
            }
        )
    return in_maps


def _run(inputs, **kw):
    from concourse.bass_utils import run_bass_kernel_spmd

    if "nc" not in _CACHE:
        _CACHE["nc"] = _build_nc()
    nc = _CACHE["nc"]
    in_maps = _prepare_in_maps(inputs)
    res = run_bass_kernel_spmd(nc, in_maps, core_ids=list(range(NCORES)), **kw)
    out = np.concatenate([np.asarray(r["out"]) for r in res.results], axis=0)
    return out.astype(np.float32), res


def kernel(**inputs):
    out, _ = _run(inputs)
    return out



# revision 2
# speedup vs baseline: 1.6874x; 1.6874x over previous
"""Linear-attention head (elu+1 feature map) on 8 TRN2 NeuronCores.

Pure data parallel: batch 16 -> 2 batches per core. Sparse-attention
compaction: the padding mask zeroes rows of phi_q/phi_k/v, and (because
S == DH) the same mask thins the O-GEMM contraction. All sequence dims
are host-compacted to C = max kept count (~531 of 1024), and the three
projection weights are row-permuted per batch to kept-first order so the
q-feature axis aligns with the compacted v-row axis of A:

  perm      = [kept_positions..., padded_positions...]
  xt[m,j]   = x[idx[j], m]                      (j < n, zero-padded to C)
  kt[d',j]  = phi_k^T in perm feature order     (pad cols forced to 0 via
                                                 rank-1 -1e9 row in PSUM)
  vt[d',j]  = (Wv_perm x + bv)*keep             (pad cols zeroed)
  qt[d',s]  = phi_q^T (pad s cols garbage; host discards)
  A[i,j]    = sum_d' vt[d',i]*kt[d',j]          (rows i>=n exactly 0)
  O[s,t]    = sum_{d'<C} qt[d',s]*A[d',t]       (exact: A rows >= n vanish)
  den[s]    = sum_{all d'} qt[d',s]*ksum[d']    (ksum = free-dim accum of kt)
  out       = O / max(den, eps), host-scattered into the full [S,S] grid.

All matmuls run in bf16 (1 cycle/row at any N, FWL weight loads); PSUM
accumulation is fp32. elu(x)+1 is computed exactly as min(exp(x),1)+relu(x).

Host-side work is layout marshalling only (mask indexing / transposes /
dtype casts).
"""

import sys

import numpy as np

if "/opt/trn_rl_repo" not in sys.path:
    sys.path.insert(0, "/opt/trn_rl_repo")

B, S, DM, DH = 16, 1024, 1024, 1024
NCORES = 8
BPC = B // NCORES  # batches per core
P = 128
NT = DM // P  # 8 tiles of 128 along the model/feature dims
NEG = -1.0e9
EPS = 1e-6

_CACHE = {}


def _build_nc(C):
    import concourse.bacc as bacc
    import concourse.bass as bass
    import concourse.mybir as mybir
    import concourse.tile as tile

    f32 = mybir.dt.float32
    bf16 = mybir.dt.bfloat16
    Act = mybir.ActivationFunctionType
    Op = mybir.AluOpType

    NTC = (C + P - 1) // P  # partition tiles over the compacted dim
    rows = [min(P, C - i * P) for i in range(NTC)]
    if C > 512:
        chunks = [(0, 512), (512, C)]
    else:
        chunks = [(0, C)]

    nc = bacc.Bacc()

    xt_ext = nc.declare_dram_parameter("xt", [BPC, DM, C], bf16, isOutput=False)
    wt_ext = nc.declare_dram_parameter("wt", [BPC, 3, NT, P, DH], bf16, isOutput=False)
    bias_ext = nc.declare_dram_parameter("bias", [BPC, P, 3 * NT], f32, isOutput=False)
    mrow_ext = nc.declare_dram_parameter("mrow", [BPC, C], bf16, isOutput=False)
    consts_ext = nc.declare_dram_parameter("consts", [2, P], bf16, isOutput=False)
    out_ext = nc.declare_dram_parameter("out", [BPC, C, C], f32, isOutput=True)

    W_Q, W_K, W_V = 0, 1, 2
    BIAS_COL = {W_Q: 0, W_K: NT, W_V: 2 * NT}

    with tile.TileContext(nc) as tc:
        with (
            tc.tile_pool(name="const", bufs=1) as cpool,
            tc.tile_pool(name="rows", bufs=2) as rpool,
            tc.tile_pool(name="keept", bufs=2) as ktpool,
            tc.tile_pool(name="tiny", bufs=3) as spool,
            tc.tile_pool(name="xt", bufs=2 * NT) as xtpool,
            tc.tile_pool(name="at", bufs=NTC + 2) as atpool,
            tc.tile_pool(name="kvq", bufs=NT + 1) as kvqpool,
            tc.tile_pool(name="wt", bufs=10) as wpool,
            tc.tile_pool(name="actE", bufs=2) as apool,
            tc.tile_pool(name="actR", bufs=2) as rrpool,
            tc.tile_pool(name="ost", bufs=2) as opool,
            tc.tile_pool(name="ps", bufs=3, space="PSUM") as pspool,
            tc.tile_pool(name="psden", bufs=2, space="PSUM") as dpool,
        ):
            # ---- constants ----
            ones_col = cpool.tile([1, P], bf16, tag="ones")
            nc.sync.dma_start(ones_col[:], consts_ext[0:1, :])
            neg_col = cpool.tile([1, P], bf16, tag="neg")
            nc.sync.dma_start(neg_col[:], consts_ext[1:2, :])

            def fence(reads, writes):
                # walrus' Matmult pseudo carries at most ONE embedded sync
                # wait. A PE NoOp declaring the group's reads/writes absorbs
                # all foreign-proc waits (NoOp carries many, like the Tile
                # tail drain), leaving each matmul's own wait count <= 1.
                eng = nc.tensor
                eng.add_instruction(
                    mybir.InstNoOp(
                        name=nc.get_next_instruction_name(),
                        text_hint="dep_fence",
                        bass_nofuse=True,
                        ins=[eng.lower_ap(a) for a in reads],
                        outs=[eng.lower_ap(a) for a in writes],
                    )
                )

            def mm_psum(reads):
                ps = pspool.tile([P, C], f32, tag="mm")
                fence(reads, [ps[:]])
                return ps

            for b in range(BPC):
                # ---- pad-mask prep (1.0 at tail cols j >= n) ----
                mrow = rpool.tile([1, C], bf16, tag="mrow")
                nc.sync.dma_start(mrow[:], mrow_ext[b : b + 1, :])
                # broadcast pad row to all 128 partitions via PE rank-1,
                # then flip (keep = 1 - pad) during the PSUM evacuation
                kb_ps = mm_psum([ones_col[:], mrow[:]])
                for c0, c1 in chunks:
                    nc.tensor.matmul(
                        kb_ps[:, c0:c1], ones_col[:], mrow[:, c0:c1],
                        start=True, stop=True,
                    )
                keep_tile = ktpool.tile([P, C], f32, tag="keeptile")
                nc.vector.tensor_scalar(
                    out=keep_tile[:], in0=kb_ps[:], scalar1=-1.0, scalar2=1.0,
                    op0=Op.mult, op1=Op.add,
                )

                bias_sb = spool.tile([P, 3 * NT], f32, tag="bias")
                nc.sync.dma_start(bias_sb[:], bias_ext[b])

                # ---- x^T tiles ----
                xt = []
                for mt in range(NT):
                    t = xtpool.tile([P, C], bf16, tag="xt")
                    nc.sync.dma_start(t[:], xt_ext[b, mt * P : (mt + 1) * P, :])
                    xt.append(t)

                # ---- projections ----
                def project(which, masked_rank1):
                    """Yields NT [128(d'), C(s)] PSUM tiles, one per dt."""
                    wts = []
                    for mt in range(NT):
                        wt = wpool.tile([P, DH], bf16, tag="wt")
                        nc.sync.dma_start(wt[:], wt_ext[b, which, mt])
                        wts.append(wt)
                    tiles = []
                    for dt in range(NT):
                        deps = [w[:] for w in wts] + [t[:] for t in xt]
                        if masked_rank1:
                            deps += [neg_col[:], mrow[:]]
                        ps = mm_psum(deps)
                        ds = slice(dt * P, (dt + 1) * P)
                        for mt in range(NT):
                            for c0, c1 in chunks:
                                nc.tensor.matmul(
                                    ps[:, c0:c1],
                                    wts[mt][:, ds],
                                    xt[mt][:, c0:c1],
                                    start=(mt == 0),
                                    stop=(mt == NT - 1) and not masked_rank1,
                                )
                        if masked_rank1:
                            for c0, c1 in chunks:
                                nc.tensor.matmul(
                                    ps[:, c0:c1], neg_col[:], mrow[:, c0:c1],
                                    start=False, stop=(c1 == chunks[-1][1]),
                                )
                        tiles.append(ps)
                    return tiles

                # K projection (perm feature order): rank-1 -1e9*pad row
                # forces phi_k at tail cols to 0
                kt = []
                ksum = spool.tile([P, NT + 1], bf16, tag="ksum")
                for dt, ps in enumerate(project(W_K, True)):
                    bcol = bias_sb[:, BIAS_COL[W_K] + dt : BIAS_COL[W_K] + dt + 1]
                    E = apool.tile([P, C], f32, tag="E")
                    nc.scalar.activation(E[:], ps[:], Act.Exp, bias=bcol)
                    R = rrpool.tile([P, C], f32, tag="R")
                    nc.vector.tensor_scalar(
                        out=R[:], in0=ps[:], scalar1=bcol, scalar2=0.0,
                        op0=Op.add, op1=Op.max,
                    )
                    t = kvqpool.tile([P, C], bf16, tag="kt")
                    nc.vector.scalar_tensor_tensor(
                        out=t[:], in0=E[:], scalar=1.0, in1=R[:],
                        op0=Op.min, op1=Op.add,
                        accum_out=ksum[:, dt : dt + 1],
                    )
                    kt.append(t)

                # V projection: (psum + bv) * keep  (zeroes tail cols)
                vt = []
                for dt, ps in enumerate(project(W_V, False)):
                    bcol = bias_sb[:, BIAS_COL[W_V] + dt : BIAS_COL[W_V] + dt + 1]
                    t = kvqpool.tile([P, C], bf16, tag="vt")
                    nc.vector.scalar_tensor_tensor(
                        out=t[:], in0=ps[:], scalar=bcol, in1=keep_tile[:],
                        op0=Op.add, op1=Op.mult,
                    )
                    vt.append(t)

                # Q projection: unmasked phi_q (tail s cols discarded on host)
                qt = []
                for dt, ps in enumerate(project(W_Q, False)):
                    bcol = bias_sb[:, BIAS_COL[W_Q] + dt : BIAS_COL[W_Q] + dt + 1]
                    E = apool.tile([P, C], f32, tag="E")
                    nc.scalar.activation(E[:], ps[:], Act.Exp, bias=bcol)
                    R = rrpool.tile([P, C], f32, tag="R")
                    nc.vector.tensor_scalar(
                        out=R[:], in0=ps[:], scalar1=bcol, scalar2=0.0,
                        op0=Op.add, op1=Op.max,
                    )
                    t = kvqpool.tile([P, C], bf16, tag="qt")
                    nc.vector.scalar_tensor_tensor(
                        out=t[:], in0=E[:], scalar=1.0, in1=R[:],
                        op0=Op.min, op1=Op.add,
                        # the den matmuls run at N=2 with a pad column of
                        # ksum that must hold real data — fill it with a
                        # q-side accum.
                        accum_out=(
                            ksum[:, NT : NT + 1] if dt == NT - 1 else None
                        ),
                    )
                    qt.append(t)

                # ---- A = V @ phi_k^T  (A[i,j], i=v col (compact), j=k col) ----
                at = []
                for it in range(NTC):
                    ri = rows[it]
                    isl = slice(it * P, it * P + ri)
                    ps = mm_psum([t[:] for t in vt] + [t[:] for t in kt])
                    for dt in range(NT):
                        for c0, c1 in chunks:
                            nc.tensor.matmul(
                                ps[:ri, c0:c1],
                                vt[dt][:, isl],
                                kt[dt][:, c0:c1],
                                start=(dt == 0), stop=(dt == NT - 1),
                            )
                    t = atpool.tile([P, C], bf16, tag="at")
                    nc.vector.tensor_copy(t[:ri, :], ps[:ri, :])
                    at.append(t)

                # ---- O = phi_q[:, :C] @ A, denom, scale, store ----
                for st in range(NTC):
                    rs = rows[st]
                    ss = slice(st * P, st * P + rs)
                    ps = pspool.tile([P, C], f32, tag="mm")
                    dps = dpool.tile([P, 2], f32, tag="den")
                    fence(
                        [t[:] for t in qt] + [t[:] for t in at] + [ksum[:]],
                        [ps[:], dps[:]],
                    )
                    for it in range(NTC):
                        ri = rows[it]
                        for c0, c1 in chunks:
                            nc.tensor.matmul(
                                ps[:rs, c0:c1],
                                qt[it][:ri, ss],
                                at[it][:ri, c0:c1],
                                start=(it == 0), stop=(it == NTC - 1),
                            )
                    for dt in range(NT):
                        nc.tensor.matmul(
                            dps[:rs, :],
                            qt[dt][:, ss],
                            ksum[:, dt : dt + 2],
                            start=(dt == 0), stop=(dt == NT - 1),
                        )
                    dsb = spool.tile([P, 1], f32, tag="dsb")
                    nc.vector.tensor_scalar(
                        out=dsb[:rs], in0=dps[:rs, 0:1], scalar1=float(EPS),
                        scalar2=None, op0=Op.max,
                    )
                    z = spool.tile([P, 1], f32, tag="z")
                    nc.vector.reciprocal(z[:rs], dsb[:rs])
                    o = opool.tile([P, C], f32, tag="ost")
                    nc.vector.tensor_scalar(
                        out=o[:rs, :], in0=ps[:rs, :], scalar1=z[:rs],
                        scalar2=None, op0=Op.mult,
                    )
                    nc.sync.dma_start(out_ext[b, ss, :], o[:rs, :])

    nc.compile()
    return nc


def _prepare_in_maps(inputs):
    import concourse.mybir as mybir

    npbf16 = mybir.dt.np(mybir.dt.bfloat16)

    x = np.asarray(inputs["x"], np.float32)
    pm = np.asarray(inputs["padding_mask"])
    W = [np.asarray(inputs[k], np.float32) for k in ("Wq", "Wk", "Wv")]
    bias = [np.asarray(inputs[k], np.float32) for k in ("bq", "bk", "bv")]

    idx_list = [np.nonzero(pm[b] != 1)[0] for b in range(B)]
    ns = [len(i) for i in idx_list]
    C = max(max(ns), 2)

    xt = np.zeros((B, DM, C), npbf16)
    wt = np.zeros((B, 3, NT, P, DH), npbf16)
    bias_t = np.zeros((B, P, 3 * NT), np.float32)
    mrow = np.zeros((B, C), npbf16)
    for b in range(B):
        idx = idx_list[b]
        n = ns[b]
        rest = np.nonzero(pm[b] == 1)[0]
        perm = np.concatenate([idx, rest])
        xt[b, :, :n] = x[b, idx, :].T.astype(npbf16)
        mrow[b, n:] = 1.0
        for w in range(3):
            wp = W[w][perm]  # [DH(d' perm), DM(m)]
            wt[b, w] = wp.T.reshape(NT, P, DH).astype(npbf16)
            bias_t[b, :, w * NT : (w + 1) * NT] = bias[w][perm].reshape(NT, P).T

    consts = np.stack(
        [np.ones(P, np.float32), np.full(P, NEG, np.float32)]
    ).astype(npbf16)

    in_maps = []
    for i in range(NCORES):
        sl = slice(BPC * i, BPC * (i + 1))
        in_maps.append(
            {
                "xt": np.ascontiguousarray(xt[sl]),
                "wt": np.ascontiguousarray(wt[sl]),
                "bias": np.ascontiguousarray(bias_t[sl]),
                "mrow": np.ascontiguousarray(mrow[sl]),
                "consts": consts,
            }
        )
    return C, ns, idx_list, in_maps


def _run(inputs, **kw):
    from concourse.bass_utils import run_bass_kernel_spmd

    C, ns, idx_list, in_maps = _prepare_in_maps(inputs)
    if C not in _CACHE:
        _CACHE[C] = _build_nc(C)
    nc = _CACHE[C]
    res = run_bass_kernel_spmd(nc, in_maps, core_ids=list(range(NCORES)), **kw)
    out = np.zeros((B, S, S), np.float32)
    for b in range(B):
        core, off = divmod(b, BPC)
        n = ns[b]
        idx = idx_list[b]
        oc = np.asarray(res.results[core]["out"])[off]
        out[b][np.ix_(idx, idx)] = oc[:n, :n]
    return out, res


def kernel(**inputs):
    out, _ = _run(inputs)
    return out


# revision 7
# speedup vs baseline: 2.1117x; 1.2514x over previous
"""Linear-attention head (elu+1 feature map) on 8 TRN2 NeuronCores.

Pure data parallel: batch 16 -> 2 batches per core. Sparse-attention
compaction: the padding mask zeroes rows of phi_q/phi_k/v, and (because
S == DH) the same mask thins the O-GEMM contraction. All sequence dims
are host-compacted to C = max kept count (~531 of 1024), and the three
projection weights are row-permuted per batch to kept-first order so the
q-feature axis aligns with the compacted v-row axis of A:

  perm      = [kept_positions..., padded_positions...]
  xt[m,j]   = x[idx[j], m]                      (j < n, zero-padded to C)
  kt[d',j]  = phi_k^T in perm feature order     (pad cols forced to 0 via
                                                 rank-1 -1e9 row in PSUM)
  vt[d',j]  = (Wv_perm x + bv)*keep             (pad cols zeroed)
  qt[d',s]  = phi_q^T (pad s cols garbage; host discards)
  A[i,j]    = sum_d' vt[d',i]*kt[d',j]          (rows i>=n exactly 0)
  O[s,t]    = sum_{d'<C} qt[d',s]*A[d',t]       (exact: A rows >= n vanish)
  den[s]    = sum_{all d'} qt[d',s]*ksum[d']    (ksum = free-dim accum of kt)
  out       = O / max(den, eps), host-scattered into the full [S,S] grid.

All matmuls run in bf16 (1 cycle/row at any N, FWL weight loads); PSUM
accumulation is fp32. elu(x)+1 is computed exactly as min(exp(x),1)+relu(x).

Host-side work is layout marshalling only (mask indexing / transposes /
dtype casts).
"""

import sys

import numpy as np

if "/opt/trn_rl_repo" not in sys.path:
    sys.path.insert(0, "/opt/trn_rl_repo")

B, S, DM, DH = 16, 1024, 1024, 1024
NCORES = 8
BPC = B // NCORES  # batches per core
P = 128
NT = DM // P  # 8 tiles of 128 along the model/feature dims
NEG = -1.0e9
EPS = 1e-6

_CACHE = {}


def _build_nc(C):
    import concourse.bacc as bacc
    import concourse.bass as bass
    import concourse.mybir as mybir
    import concourse.tile as tile

    f32 = mybir.dt.float32
    bf16 = mybir.dt.bfloat16
    Act = mybir.ActivationFunctionType
    Op = mybir.AluOpType

    NTC = (C + P - 1) // P  # partition tiles over the compacted dim
    rows = [min(P, C - i * P) for i in range(NTC)]
    if C > 512:
        chunks = [(0, 512), (512, C)]
    else:
        chunks = [(0, C)]

    nc = bacc.Bacc()

    xt_ext = nc.declare_dram_parameter("xt", [BPC, DM, C], bf16, isOutput=False)
    wt_ext = nc.declare_dram_parameter("wt", [BPC, 3, NT, P, DH], bf16, isOutput=False)
    bias_ext = nc.declare_dram_parameter("bias", [BPC, P, 3 * NT], f32, isOutput=False)
    mrow_ext = nc.declare_dram_parameter("mrow", [BPC, C], bf16, isOutput=False)
    consts_ext = nc.declare_dram_parameter("consts", [2, P], bf16, isOutput=False)
    out_ext = nc.declare_dram_parameter("out", [BPC, C, C], f32, isOutput=True)

    W_Q, W_K, W_V = 0, 1, 2
    BIAS_COL = {W_Q: 0, W_K: NT, W_V: 2 * NT}

    with tile.TileContext(nc) as tc:
        with (
            tc.tile_pool(name="const", bufs=1) as cpool,
            tc.tile_pool(name="rows", bufs=2) as rpool,
            tc.tile_pool(name="keept", bufs=2) as ktpool,
            tc.tile_pool(name="tiny", bufs=3) as spool,
            tc.tile_pool(name="xt", bufs=2 * NT) as xtpool,
            tc.tile_pool(name="at", bufs=NTC + 2) as atpool,
            tc.tile_pool(name="kvq", bufs=NT + 1) as kvqpool,
            tc.tile_pool(name="wt", bufs=24) as wpool,
            tc.tile_pool(name="actE", bufs=2) as apool,
            tc.tile_pool(name="actR", bufs=2) as rrpool,
            tc.tile_pool(name="ost", bufs=2) as opool,
            tc.tile_pool(name="ps", bufs=3, space="PSUM") as pspool,
            tc.tile_pool(name="psden", bufs=2, space="PSUM") as dpool,
        ):
            # ---- constants ----
            ones_col = cpool.tile([1, P], bf16, tag="ones")
            nc.sync.dma_start(ones_col[:], consts_ext[0:1, :])
            neg_col = cpool.tile([1, P], bf16, tag="neg")
            nc.sync.dma_start(neg_col[:], consts_ext[1:2, :])

            def fence(reads, writes):
                # walrus' Matmult pseudo carries at most ONE embedded sync
                # wait. A PE NoOp declaring the group's reads/writes absorbs
                # all foreign-proc waits (NoOp carries many, like the Tile
                # tail drain), leaving each matmul's own wait count <= 1.
                eng = nc.tensor
                eng.add_instruction(
                    mybir.InstNoOp(
                        name=nc.get_next_instruction_name(),
                        text_hint="dep_fence",
                        bass_nofuse=True,
                        ins=[eng.lower_ap(a) for a in reads],
                        outs=[eng.lower_ap(a) for a in writes],
                    )
                )

            def mm_psum(reads):
                ps = pspool.tile([P, C], f32, tag="mm")
                fence(reads, [ps[:]])
                return ps

            for b in range(BPC):
                # ---- pad-mask prep (1.0 at tail cols j >= n) ----
                mrow = rpool.tile([1, C], bf16, tag="mrow")
                nc.sync.dma_start(mrow[:], mrow_ext[b : b + 1, :])
                # broadcast pad row to all 128 partitions via PE rank-1,
                # then flip (keep = 1 - pad) during the PSUM evacuation
                kb_ps = mm_psum([ones_col[:], mrow[:]])
                for c0, c1 in chunks:
                    nc.tensor.matmul(
                        kb_ps[:, c0:c1], ones_col[:], mrow[:, c0:c1],
                        start=True, stop=True,
                    )
                keep_tile = ktpool.tile([P, C], f32, tag="keeptile")
                nc.vector.tensor_scalar(
                    out=keep_tile[:], in0=kb_ps[:], scalar1=-1.0, scalar2=1.0,
                    op0=Op.mult, op1=Op.add,
                )

                bias_sb = spool.tile([P, 3 * NT], f32, tag="bias")
                nc.sync.dma_start(bias_sb[:], bias_ext[b])

                # ---- x^T tiles ----
                xt = []
                for mt in range(NT):
                    t = xtpool.tile([P, C], bf16, tag="xt")
                    nc.sync.dma_start(t[:], xt_ext[b, mt * P : (mt + 1) * P, :])
                    xt.append(t)

                # ---- projections ----
                def project(which, masked_rank1):
                    """Yields NT [128(d'), C(s)] PSUM tiles, one per dt."""
                    wts = []
                    for mt in range(NT):
                        wt = wpool.tile([P, DH], bf16, tag="wt")
                        # gpsimd queue: weight prefetch must not serialize
                        # behind x/out transfers on the sync queue
                        nc.gpsimd.dma_start(wt[:], wt_ext[b, which, mt])
                        wts.append(wt)
                    tiles = []
                    for dt in range(NT):
                        # weight tiles deliberately NOT in the fence: each
                        # matmul carries its own single wt-DMA wait, so the
                        # group starts as soon as wts[0] lands.
                        deps = [t[:] for t in xt]
                        if masked_rank1:
                            deps += [neg_col[:], mrow[:]]
                        ps = mm_psum(deps)
                        ds = slice(dt * P, (dt + 1) * P)
                        for mt in range(NT):
                            for c0, c1 in chunks:
                                nc.tensor.matmul(
                                    ps[:, c0:c1],
                                    wts[mt][:, ds],
                                    xt[mt][:, c0:c1],
                                    start=(mt == 0),
                                    stop=(mt == NT - 1) and not masked_rank1,
                                )
                        if masked_rank1:
                            for c0, c1 in chunks:
                                nc.tensor.matmul(
                                    ps[:, c0:c1], neg_col[:], mrow[:, c0:c1],
                                    start=False, stop=(c1 == chunks[-1][1]),
                                )
                        tiles.append(ps)
                    return tiles

                # K projection (perm feature order): rank-1 -1e9*pad row
                # forces phi_k at tail cols to 0
                kt = []
                ksum = spool.tile([P, NT + 1], bf16, tag="ksum")
                for dt, ps in enumerate(project(W_K, True)):
                    bcol = bias_sb[:, BIAS_COL[W_K] + dt : BIAS_COL[W_K] + dt + 1]
                    E = apool.tile([P, C], f32, tag="E")
                    nc.scalar.activation(E[:], ps[:], Act.Exp, bias=bcol)
                    R = rrpool.tile([P, C], f32, tag="R")
                    nc.scalar.activation(R[:], ps[:], Act.Relu, bias=bcol)
                    t = kvqpool.tile([P, C], bf16, tag="kt")
                    nc.vector.scalar_tensor_tensor(
                        out=t[:], in0=E[:], scalar=1.0, in1=R[:],
                        op0=Op.min, op1=Op.add,
                        accum_out=ksum[:, dt : dt + 1],
                    )
                    kt.append(t)

                # V projection: (psum + bv) * keep  (zeroes tail cols)
                vt = []
                for dt, ps in enumerate(project(W_V, False)):
                    bcol = bias_sb[:, BIAS_COL[W_V] + dt : BIAS_COL[W_V] + dt + 1]
                    t = kvqpool.tile([P, C], bf16, tag="vt")
                    nc.vector.scalar_tensor_tensor(
                        out=t[:], in0=ps[:], scalar=bcol, in1=keep_tile[:],
                        op0=Op.add, op1=Op.mult,
                    )
                    vt.append(t)

                # Q projection: unmasked phi_q (tail s cols discarded on host)
                qt = []
                for dt, ps in enumerate(project(W_Q, False)):
                    bcol = bias_sb[:, BIAS_COL[W_Q] + dt : BIAS_COL[W_Q] + dt + 1]
                    E = apool.tile([P, C], f32, tag="E")
                    nc.scalar.activation(E[:], ps[:], Act.Exp, bias=bcol)
                    R = rrpool.tile([P, C], f32, tag="R")
                    nc.scalar.activation(R[:], ps[:], Act.Relu, bias=bcol)
                    t = kvqpool.tile([P, C], bf16, tag="qt")
                    nc.vector.scalar_tensor_tensor(
                        out=t[:], in0=E[:], scalar=1.0, in1=R[:],
                        op0=Op.min, op1=Op.add,
                        # the den matmuls run at N=2 with a pad column of
                        # ksum that must hold real data — fill it with a
                        # q-side accum.
                        accum_out=(
                            ksum[:, NT : NT + 1] if dt == NT - 1 else None
                        ),
                    )
                    qt.append(t)

                # ---- A = V @ phi_k^T  (A[i,j], i=v col (compact), j=k col) ----
                at = []
                for it in range(NTC):
                    ri = rows[it]
                    isl = slice(it * P, it * P + ri)
                    ps = mm_psum([t[:] for t in vt] + [t[:] for t in kt])
                    for dt in range(NT):
                        for c0, c1 in chunks:
                            nc.tensor.matmul(
                                ps[:ri, c0:c1],
                                vt[dt][:, isl],
                                kt[dt][:, c0:c1],
                                start=(dt == 0), stop=(dt == NT - 1),
                            )
                    t = atpool.tile([P, C], bf16, tag="at")
                    nc.vector.tensor_copy(t[:ri, :], ps[:ri, :])
                    at.append(t)

                # ---- denominator (only needs qt + ksum; runs while the at
                # evacuations drain so the O loop ships output immediately) ----
                zs = []
                for st in range(NTC):
                    rs = rows[st]
                    ss = slice(st * P, st * P + rs)
                    dps = dpool.tile([P, 2], f32, tag="den")
                    fence([t[:] for t in qt] + [ksum[:]], [dps[:]])
                    for dt in range(NT):
                        nc.tensor.matmul(
                            dps[:rs, :],
                            qt[dt][:, ss],
                            ksum[:, dt : dt + 2],
                            start=(dt == 0), stop=(dt == NT - 1),
                        )
                    dsb = spool.tile([P, 1], f32, tag="dsb")
                    nc.vector.tensor_scalar(
                        out=dsb[:rs], in0=dps[:rs, 0:1], scalar1=float(EPS),
                        scalar2=None, op0=Op.max,
                    )
                    z = spool.tile([P, 1], f32, tag="z", bufs=NTC + 1)
                    nc.vector.reciprocal(z[:rs], dsb[:rs])
                    zs.append(z)

                # ---- O = phi_q[:, :C] @ A, scale, store ----
                for st in range(NTC):
                    rs = rows[st]
                    ss = slice(st * P, st * P + rs)
                    ps = pspool.tile([P, C], f32, tag="mm")
                    fence([t[:] for t in qt] + [t[:] for t in at], [ps[:]])
                    for it in range(NTC):
                        ri = rows[it]
                        for c0, c1 in chunks:
                            nc.tensor.matmul(
                                ps[:rs, c0:c1],
                                qt[it][:ri, ss],
                                at[it][:ri, c0:c1],
                                start=(it == 0), stop=(it == NTC - 1),
                            )
                    o = opool.tile([P, C], f32, tag="ost")
                    nc.vector.tensor_scalar(
                        out=o[:rs, :], in0=ps[:rs, :], scalar1=zs[st][:rs],
                        scalar2=None, op0=Op.mult,
                    )
                    nc.sync.dma_start(out_ext[b, ss, :], o[:rs, :])

    nc.compile()
    return nc


def _prepare_in_maps(inputs):
    import concourse.mybir as mybir

    npbf16 = mybir.dt.np(mybir.dt.bfloat16)

    x = np.asarray(inputs["x"], np.float32)
    pm = np.asarray(inputs["padding_mask"])
    W = [np.asarray(inputs[k], np.float32) for k in ("Wq", "Wk", "Wv")]
    bias = [np.asarray(inputs[k], np.float32) for k in ("bq", "bk", "bv")]

    idx_list = [np.nonzero(pm[b] != 1)[0] for b in range(B)]
    ns = [len(i) for i in idx_list]
    C = max(max(ns), 2)

    xt = np.zeros((B, DM, C), npbf16)
    wt = np.zeros((B, 3, NT, P, DH), npbf16)
    bias_t = np.zeros((B, P, 3 * NT), np.float32)
    mrow = np.zeros((B, C), npbf16)
    for b in range(B):
        idx = idx_list[b]
        n = ns[b]
        rest = np.nonzero(pm[b] == 1)[0]
        perm = np.concatenate([idx, rest])
        xt[b, :, :n] = x[b, idx, :].T.astype(npbf16)
        mrow[b, n:] = 1.0
        for w in range(3):
            wp = W[w][perm]  # [DH(d' perm), DM(m)]
            wt[b, w] = wp.T.reshape(NT, P, DH).astype(npbf16)
            bias_t[b, :, w * NT : (w + 1) * NT] = bias[w][perm].reshape(NT, P).T

    consts = np.stack(
        [np.ones(P, np.float32), np.full(P, NEG, np.float32)]
    ).astype(npbf16)

    in_maps = []
    for i in range(NCORES):
        sl = slice(BPC * i, BPC * (i + 1))
        in_maps.append(
            {
                "xt": np.ascontiguousarray(xt[sl]),
                "wt": np.ascontiguousarray(wt[sl]),
                "bias": np.ascontiguousarray(bias_t[sl]),
                "mrow": np.ascontiguousarray(mrow[sl]),
                "consts": consts,
            }
        )
    return C, ns, idx_list, in_maps


def _run(inputs, **kw):
    from concourse.bass_utils import run_bass_kernel_spmd

    C, ns, idx_list, in_maps = _prepare_in_maps(inputs)
    if C not in _CACHE:
        _CACHE[C] = _build_nc(C)
    nc = _CACHE[C]
    res = run_bass_kernel_spmd(nc, in_maps, core_ids=list(range(NCORES)), **kw)
    out = np.zeros((B, S, S), np.float32)
    for b in range(B):
        core, off = divmod(b, BPC)
        n = ns[b]
        idx = idx_list[b]
        oc = np.asarray(res.results[core]["out"])[off]
        out[b][np.ix_(idx, idx)] = oc[:n, :n]
    return out, res


def kernel(**inputs):
    out, _ = _run(inputs)
    return out


# revision 12
# speedup vs baseline: 2.1238x; 1.0057x over previous
"""Linear-attention head (elu+1 feature map) on 8 TRN2 NeuronCores.

Pure data parallel: batch 16 -> 2 batches per core. Sparse-attention
compaction: the padding mask zeroes rows of phi_q/phi_k/v, and (because
S == DH) the same mask thins the O-GEMM contraction. All sequence dims
are host-compacted to C = max kept count (~531 of 1024), and the three
projection weights are row-permuted per batch to kept-first order so the
q-feature axis aligns with the compacted v-row axis of A:

  perm      = [kept_positions..., padded_positions...]
  xt[m,j]   = x[idx[j], m]                      (j < n, zero-padded to C)
  kt[d',j]  = phi_k^T in perm feature order     (pad cols forced to 0 via
                                                 rank-1 -1e9 row in PSUM)
  vt[d',j]  = (Wv_perm x + bv)*keep             (pad cols zeroed)
  qt[d',s]  = phi_q^T (pad s cols garbage; host discards)
  A[i,j]    = sum_d' vt[d',i]*kt[d',j]          (rows i>=n exactly 0)
  O[s,t]    = sum_{d'<C} qt[d',s]*A[d',t]       (exact: A rows >= n vanish)
  den[s]    = sum_{all d'} qt[d',s]*ksum[d']    (ksum = free-dim accum of kt)
  out       = O / max(den, eps), host-scattered into the full [S,S] grid.

All matmuls run in bf16 (1 cycle/row at any N, FWL weight loads); PSUM
accumulation is fp32. elu(x)+1 is computed exactly as min(exp(x),1)+relu(x).

Host-side work is layout marshalling only (mask indexing / transposes /
dtype casts).
"""

import sys

import numpy as np

if "/opt/trn_rl_repo" not in sys.path:
    sys.path.insert(0, "/opt/trn_rl_repo")

B, S, DM, DH = 16, 1024, 1024, 1024
NCORES = 8
BPC = B // NCORES  # batches per core
P = 128
NT = DM // P  # 8 tiles of 128 along the model/feature dims
NEG = -1.0e9
EPS = 1e-6

_CACHE = {}


def _build_nc(C):
    import concourse.bacc as bacc
    import concourse.bass as bass
    import concourse.mybir as mybir
    import concourse.tile as tile

    f32 = mybir.dt.float32
    bf16 = mybir.dt.bfloat16
    Act = mybir.ActivationFunctionType
    Op = mybir.AluOpType

    NTC = (C + P - 1) // P  # partition tiles over the compacted dim
    rows = [min(P, C - i * P) for i in range(NTC)]
    if C > 512:
        chunks = [(0, 512), (512, C)]
    else:
        chunks = [(0, C)]

    nc = bacc.Bacc()

    xt_ext = nc.declare_dram_parameter("xt", [BPC, DM, C], bf16, isOutput=False)
    wt_ext = nc.declare_dram_parameter("wt", [BPC, 3, NT, P, DH], bf16, isOutput=False)
    bias_ext = nc.declare_dram_parameter("bias", [BPC, P, 3 * NT], f32, isOutput=False)
    mrow_ext = nc.declare_dram_parameter("mrow", [BPC, C], bf16, isOutput=False)
    consts_ext = nc.declare_dram_parameter("consts", [2, P], bf16, isOutput=False)
    out_ext = nc.declare_dram_parameter("out", [BPC, C, C], bf16, isOutput=True)

    W_Q, W_K, W_V = 0, 1, 2
    BIAS_COL = {W_Q: 0, W_K: NT, W_V: 2 * NT}

    with tile.TileContext(nc) as tc:
        with (
            tc.tile_pool(name="const", bufs=1) as cpool,
            tc.tile_pool(name="rows", bufs=2) as rpool,
            tc.tile_pool(name="keept", bufs=2) as ktpool,
            tc.tile_pool(name="tiny", bufs=3) as spool,
            tc.tile_pool(name="xt", bufs=2 * NT) as xtpool,
            tc.tile_pool(name="at", bufs=NTC + 2) as atpool,
            tc.tile_pool(name="kvq", bufs=NT + 1) as kvqpool,
            tc.tile_pool(name="wt", bufs=6 * NT) as wpool,
            tc.tile_pool(name="actE", bufs=2) as apool,
            tc.tile_pool(name="actR", bufs=2) as rrpool,
            tc.tile_pool(name="ost", bufs=2) as opool,
            tc.tile_pool(name="ps", bufs=3, space="PSUM") as pspool,
            tc.tile_pool(name="psden", bufs=2, space="PSUM") as dpool,
        ):
            # ---- constants ----
            ones_col = cpool.tile([1, P], bf16, tag="ones")
            nc.sync.dma_start(ones_col[:], consts_ext[0:1, :])
            neg_col = cpool.tile([1, P], bf16, tag="neg")
            nc.sync.dma_start(neg_col[:], consts_ext[1:2, :])

            def fence(reads, writes):
                # walrus' Matmult pseudo carries at most ONE embedded sync
                # wait. A PE NoOp declaring the group's reads/writes absorbs
                # all foreign-proc waits (NoOp carries many, like the Tile
                # tail drain), leaving each matmul's own wait count <= 1.
                eng = nc.tensor
                eng.add_instruction(
                    mybir.InstNoOp(
                        name=nc.get_next_instruction_name(),
                        text_hint="dep_fence",
                        bass_nofuse=True,
                        ins=[eng.lower_ap(a) for a in reads],
                        outs=[eng.lower_ap(a) for a in writes],
                    )
                )

            def mm_psum(reads):
                ps = pspool.tile([P, C], f32, tag="mm")
                fence(reads, [ps[:]])
                return ps

            # ---- prologue: issue ALL input DMAs up front. Full double
            # buffering (no ring reuse) means no WAR waits; the two queues
            # (sync: x/masks, gpsimd: weights) stream while PE computes. ----
            pre = []
            for b in range(BPC):
                d = {}
                mrow = rpool.tile([1, C], bf16, tag="mrow")
                nc.sync.dma_start(mrow[:], mrow_ext[b : b + 1, :])
                d["mrow"] = mrow
                bias_sb = spool.tile([P, 3 * NT], f32, tag="bias")
                nc.sync.dma_start(bias_sb[:], bias_ext[b])
                d["bias"] = bias_sb
                d["xt"] = []
                for mt in range(NT):
                    t = xtpool.tile([P, C], bf16, tag="xt")
                    nc.sync.dma_start(t[:], xt_ext[b, mt * P : (mt + 1) * P, :])
                    d["xt"].append(t)
                d["wt"] = {}
                for which in (1, 2, 0):  # consumption order K, V, Q
                    wts = []
                    for mt in range(NT):
                        wt = wpool.tile([P, DH], bf16, tag="wt")
                        nc.gpsimd.dma_start(wt[:], wt_ext[b, which, mt])
                        wts.append(wt)
                    d["wt"][which] = wts
                pre.append(d)

            for b in range(BPC):
                mrow = pre[b]["mrow"]
                bias_sb = pre[b]["bias"]
                xt = pre[b]["xt"]
                # ---- pad-mask prep (1.0 at tail cols j >= n) ----
                # broadcast pad row to all 128 partitions via PE rank-1,
                # then flip (keep = 1 - pad) during the PSUM evacuation
                kb_ps = mm_psum([ones_col[:], mrow[:]])
                for c0, c1 in chunks:
                    nc.tensor.matmul(
                        kb_ps[:, c0:c1], ones_col[:], mrow[:, c0:c1],
                        start=True, stop=True,
                    )
                keep_tile = ktpool.tile([P, C], f32, tag="keeptile")
                nc.vector.tensor_scalar(
                    out=keep_tile[:], in0=kb_ps[:], scalar1=-1.0, scalar2=1.0,
                    op0=Op.mult, op1=Op.add,
                )

                # ---- projections ----
                def project(which, masked_rank1):
                    """Yields NT [128(d'), C(s)] PSUM tiles, one per dt."""
                    wts = pre[b]["wt"][which]
                    tiles = []
                    for dt in range(NT):
                        # x/weight tiles deliberately NOT in the fence: each
                        # matmul carries its own DMA waits, so the group
                        # starts as soon as the first tiles land.
                        deps = []
                        if masked_rank1:
                            deps += [neg_col[:], mrow[:]]
                        ps = mm_psum(deps)
                        ds = slice(dt * P, (dt + 1) * P)
                        for mt in range(NT):
                            for c0, c1 in chunks:
                                nc.tensor.matmul(
                                    ps[:, c0:c1],
                                    wts[mt][:, ds],
                                    xt[mt][:, c0:c1],
                                    start=(mt == 0),
                                    stop=(mt == NT - 1) and not masked_rank1,
                                )
                        if masked_rank1:
                            for c0, c1 in chunks:
                                nc.tensor.matmul(
                                    ps[:, c0:c1], neg_col[:], mrow[:, c0:c1],
                                    start=False, stop=(c1 == chunks[-1][1]),
                                )
                        tiles.append(ps)
                    return tiles

                # K projection (perm feature order): rank-1 -1e9*pad row
                # forces phi_k at tail cols to 0
                kt = []
                ksum = spool.tile([P, NT + 1], bf16, tag="ksum")
                for dt, ps in enumerate(project(W_K, True)):
                    bcol = bias_sb[:, BIAS_COL[W_K] + dt : BIAS_COL[W_K] + dt + 1]
                    E = apool.tile([P, C], f32, tag="E")
                    nc.scalar.activation(E[:], ps[:], Act.Exp, bias=bcol)
                    R = rrpool.tile([P, C], f32, tag="R")
                    nc.scalar.activation(R[:], ps[:], Act.Relu, bias=bcol)
                    t = kvqpool.tile([P, C], bf16, tag="kt")
                    nc.vector.scalar_tensor_tensor(
                        out=t[:], in0=E[:], scalar=1.0, in1=R[:],
                        op0=Op.min, op1=Op.add,
                        accum_out=ksum[:, dt : dt + 1],
                    )
                    kt.append(t)

                # V projection: (psum + bv) * keep  (zeroes tail cols)
                vt = []
                for dt, ps in enumerate(project(W_V, False)):
                    bcol = bias_sb[:, BIAS_COL[W_V] + dt : BIAS_COL[W_V] + dt + 1]
                    t = kvqpool.tile([P, C], bf16, tag="vt")
                    nc.vector.scalar_tensor_tensor(
                        out=t[:], in0=ps[:], scalar=bcol, in1=keep_tile[:],
                        op0=Op.add, op1=Op.mult,
                    )
                    vt.append(t)

                # Q projection: unmasked phi_q (tail s cols discarded on host)
                qt = []
                for dt, ps in enumerate(project(W_Q, False)):
                    bcol = bias_sb[:, BIAS_COL[W_Q] + dt : BIAS_COL[W_Q] + dt + 1]
                    E = apool.tile([P, C], f32, tag="E")
                    nc.scalar.activation(E[:], ps[:], Act.Exp, bias=bcol)
                    R = rrpool.tile([P, C], f32, tag="R")
                    nc.scalar.activation(R[:], ps[:], Act.Relu, bias=bcol)
                    t = kvqpool.tile([P, C], bf16, tag="qt")
                    nc.vector.scalar_tensor_tensor(
                        out=t[:], in0=E[:], scalar=1.0, in1=R[:],
                        op0=Op.min, op1=Op.add,
                        # the den matmuls run at N=2 with a pad column of
                        # ksum that must hold real data — fill it with a
                        # q-side accum.
                        accum_out=(
                            ksum[:, NT : NT + 1] if dt == NT - 1 else None
                        ),
                    )
                    qt.append(t)

                # ---- A = V @ phi_k^T  (A[i,j], i=v col (compact), j=k col) ----
                at = []
                for it in range(NTC):
                    ri = rows[it]
                    isl = slice(it * P, it * P + ri)
                    ps = mm_psum([t[:] for t in vt] + [t[:] for t in kt])
                    for dt in range(NT):
                        for c0, c1 in chunks:
                            nc.tensor.matmul(
                                ps[:ri, c0:c1],
                                vt[dt][:, isl],
                                kt[dt][:, c0:c1],
                                start=(dt == 0), stop=(dt == NT - 1),
                            )
                    t = atpool.tile([P, C], bf16, tag="at")
                    nc.vector.tensor_copy(t[:ri, :], ps[:ri, :])
                    at.append(t)

                # ---- denominator (only needs qt + ksum; runs while the at
                # evacuations drain so the O loop ships output immediately) ----
                zs = []
                for st in range(NTC):
                    rs = rows[st]
                    ss = slice(st * P, st * P + rs)
                    dps = dpool.tile([P, 2], f32, tag="den")
                    fence([t[:] for t in qt] + [ksum[:]], [dps[:]])
                    for dt in range(NT):
                        nc.tensor.matmul(
                            dps[:rs, :],
                            qt[dt][:, ss],
                            ksum[:, dt : dt + 2],
                            start=(dt == 0), stop=(dt == NT - 1),
                        )
                    dsb = spool.tile([P, 1], f32, tag="dsb")
                    nc.vector.tensor_scalar(
                        out=dsb[:rs], in0=dps[:rs, 0:1], scalar1=float(EPS),
                        scalar2=None, op0=Op.max,
                    )
                    z = spool.tile([P, 1], f32, tag="z", bufs=NTC + 1)
                    nc.vector.reciprocal(z[:rs], dsb[:rs])
                    zs.append(z)

                # ---- O = phi_q[:, :C] @ A, scale, store ----
                for st in range(NTC):
                    rs = rows[st]
                    ss = slice(st * P, st * P + rs)
                    ps = pspool.tile([P, C], f32, tag="mm")
                    fence([t[:] for t in qt] + [t[:] for t in at], [ps[:]])
                    for it in range(NTC):
                        ri = rows[it]
                        for c0, c1 in chunks:
                            nc.tensor.matmul(
                                ps[:rs, c0:c1],
                                qt[it][:ri, ss],
                                at[it][:ri, c0:c1],
                                start=(it == 0), stop=(it == NTC - 1),
                            )
                    o = opool.tile([P, C], bf16, tag="ost")
                    nc.vector.tensor_scalar(
                        out=o[:rs, :], in0=ps[:rs, :], scalar1=zs[st][:rs],
                        scalar2=None, op0=Op.mult,
                    )
                    nc.sync.dma_start(out_ext[b, ss, :], o[:rs, :])

    nc.compile()
    return nc


def _prepare_in_maps(inputs):
    import concourse.mybir as mybir

    npbf16 = mybir.dt.np(mybir.dt.bfloat16)

    x = np.asarray(inputs["x"], np.float32)
    pm = np.asarray(inputs["padding_mask"])
    W = [np.asarray(inputs[k], np.float32) for k in ("Wq", "Wk", "Wv")]
    bias = [np.asarray(inputs[k], np.float32) for k in ("bq", "bk", "bv")]

    idx_list = [np.nonzero(pm[b] != 1)[0] for b in range(B)]
    ns = [len(i) for i in idx_list]
    C = max(max(ns), 2)

    xt = np.zeros((B, DM, C), npbf16)
    wt = np.zeros((B, 3, NT, P, DH), npbf16)
    bias_t = np.zeros((B, P, 3 * NT), np.float32)
    mrow = np.zeros((B, C), npbf16)
    for b in range(B):
        idx = idx_list[b]
        n = ns[b]
        rest = np.nonzero(pm[b] == 1)[0]
        perm = np.concatenate([idx, rest])
        xt[b, :, :n] = x[b, idx, :].T.astype(npbf16)
        mrow[b, n:] = 1.0
        for w in range(3):
            wp = W[w][perm]  # [DH(d' perm), DM(m)]
            wt[b, w] = wp.T.reshape(NT, P, DH).astype(npbf16)
            bias_t[b, :, w * NT : (w + 1) * NT] = bias[w][perm].reshape(NT, P).T

    consts = np.stack(
        [np.ones(P, np.float32), np.full(P, NEG, np.float32)]
    ).astype(npbf16)

    in_maps = []
    for i in range(NCORES):
        sl = slice(BPC * i, BPC * (i + 1))
        in_maps.append(
            {
                "xt": np.ascontiguousarray(xt[sl]),
                "wt": np.ascontiguousarray(wt[sl]),
                "bias": np.ascontiguousarray(bias_t[sl]),
                "mrow": np.ascontiguousarray(mrow[sl]),
                "consts": consts,
            }
        )
    return C, ns, idx_list, in_maps


def _run(inputs, **kw):
    from concourse.bass_utils import run_bass_kernel_spmd

    C, ns, idx_list, in_maps = _prepare_in_maps(inputs)
    if C not in _CACHE:
        _CACHE[C] = _build_nc(C)
    nc = _CACHE[C]
    res = run_bass_kernel_spmd(nc, in_maps, core_ids=list(range(NCORES)), **kw)
    out = np.zeros((B, S, S), np.float32)
    for b in range(B):
        core, off = divmod(b, BPC)
        n = ns[b]
        idx = idx_list[b]
        oc = np.asarray(res.results[core]["out"])[off].astype(np.float32)
        out[b][np.ix_(idx, idx)] = oc[:n, :n]
    return out, res


def kernel(**inputs):
    out, _ = _run(inputs)
    return out
